# revision 1
# baseline (speedup 1.0000x reference)
"""ConvDecoder Bass kernel for Trainium2, SPMD over 8 NeuronCores.

Math (per batch element b, one per core):
    r_conv = Conv1d(r, conv_w, SAME) + conv_b            # (C, N_IN)
    d[n,m] = (xc[n] - xt[m])^2                           # (N_IN, N_OUT)
    wt_c   = exp(-0.5 * d / exp(sigma_c)^2)
    z[m,c] = sum_n r_conv[c,n] * wt_c[n,m]
    out    = z @ lin_w.T + lin_b                         # (N_OUT, OUT_C)

Per-core structure (v2):
  - Conv1d as an im2col matmul: ones row (bias) + 5 shifted DMA copies of r
    stacked on partitions -> (81, 512); matmul with repacked weights
    (81, 16) yields the conv output directly in (n, c) layout = the lhsT
    of the RBF-reduction matmul.
  - xt is partition-broadcast by a stride-0 DMA; GpSimd computes
    diff = xt - xc_p (per-partition scalar), DVE/ACT square it, ACT
    exponentiates with scale=-a (a = 0.5/scale^2, baked per group) ->
    E chunk (128, 512). fp32 end to end.
  - z[c,m] over the 4 n-tiles: 4 matmuls issued to 4 distinct PE column
    strips (tile_position) run concurrently in the array; DVE reduces the
    4 PSUM partials. (Channels sharing a length scale share one E map;
    with >1 sigma group, falls back to sequential PSUM accumulation.)
  - Final linear: (16,128)^T @ (16,32) matmul per m-tile; lin_b is folded
    into the PSUM->SBUF copy as a DVE add against a broadcast row.
"""

import numpy as np

import concourse.bass as bass
import concourse.mybir as mybir
from concourse.tile import TileContext
from concourse.bass_utils import run_bass_kernel_spmd

F32 = mybir.dt.float32

B, N_IN, N_OUT, C, OUT_C, KW = 8, 512, 1024, 16, 32, 5
N_CORES = 8
NT = N_IN // 128   # n tiles (4)
MH = N_OUT // 512  # m halves (2)
MT = 512 // 128    # m tiles per half (4)

# chunks (mh*NT+k) whose sub+square run fused on ACT (Square with
# per-partition bias) instead of DVE, to balance engine load against
# ACT's exp passes.
SQ_ON_ACT = {3, 7}


# --- walrus workaround -----------------------------------------------------
# This container's walrus accepts at most ONE semaphore wait per TPB
# instruction, but Tile's scheduler attaches several (joins + tail drain).
# Hoist all but the last wait of each instruction onto fresh wait-only
# EventSemaphore instructions inserted right before it on the same engine.
_ws_ctr = [0]


def _split_multi_waits(nc):
    for fn in nc.m.functions:
        for blk in fn.blocks:
            insts = blk.instructions
            if not any(
                ins.sync_info and len(ins.sync_info.on_wait) > 1 for ins in insts
            ):
                continue
            out = []
            for ins in insts:
                si = ins.sync_info
                waits = list(si.on_wait) if si else []
                if len(waits) > 1:
                    for w in waits[:-1]:
                        _ws_ctr[0] += 1
                        ev = mybir.InstEventSemaphore(
                            name=f"waitsplit_{_ws_ctr[0]}", ins=[], outs=[]
                        )
                        ev.engine = ins.engine
                        ev.sync_info = mybir.SyncInfo(on_wait=[w], on_update=[])
                        nc.register_instruction(ev)
                        out.append(ev)
                    ins.sync_info = mybir.SyncInfo(
                        on_wait=[waits[-1]], on_update=list(si.on_update)
                    )
                out.append(ins)
            insts[:] = out


# --- kernel build ----------------------------------------------------------
def _build(groups):
    """groups: tuple of (c0, c1, a) with contiguous channel ranges."""
    nc = bass.Bass()
    r_in = nc.dram_tensor("r", [C, N_IN], F32, kind="ExternalInput")
    xc_in = nc.dram_tensor("xc", [1, N_IN], F32, kind="ExternalInput")
    xt_in = nc.dram_tensor("xt", [1, N_OUT], F32, kind="ExternalInput")
    wconv = nc.dram_tensor("w_aug", [C * KW + 1, C], F32, kind="ExternalInput")
    # lin128: lin_w_t at rows 32j+c, zeros elsewhere — the final matmul
    # contracts over the 4 z-strip partials and 16 channels in one go
    # (matmul cost is N-bound, so the 128-row contraction is free).
    wlin = nc.dram_tensor("lin128", [128, OUT_C], F32, kind="ExternalInput")
    blin = nc.dram_tensor("lin_b", [1, OUT_C], F32, kind="ExternalInput")
    y_out = nc.dram_tensor("y", [N_OUT, OUT_C], F32, kind="ExternalOutput")

    Exp = mybir.ActivationFunctionType.Exp
    Square = mybir.ActivationFunctionType.Square
    single_group = len(groups) == 1

    with TileContext(nc) as tc:
        with (
            tc.tile_pool(name="const", bufs=1) as cpool,
            tc.tile_pool(name="work", bufs=1) as wpool,
            tc.tile_pool(name="psum", bufs=1, space="PSUM") as ppool,
        ):
            # ---- inputs on the critical path first ----
            # xc as per-partition scalars: xc_pt[p, t] = xc[t*128 + p]
            xc_pt = cpool.tile([128, NT], F32)
            nc.sync.dma_start(
                out=xc_pt[:], in_=xc_in[0, :].rearrange("(t p) -> p t", p=128)
            )
            # xt broadcast to all partitions, one tile per m-half
            xtb = []
            for mh in range(MH):
                t = cpool.tile([128, 512], F32, name=f"xtb{mh}")
                nc.sync.dma_start(
                    out=t[:],
                    in_=xt_in[0:1, mh * 512 : (mh + 1) * 512].partition_broadcast(128),
                )
                xtb.append(t)
            # dummy exp: forces the ~1.3us ACT table load to run at t~=0
            # instead of stalling the first real exp mid-pipeline
            warm = cpool.tile([128, NT], F32)
            nc.scalar.activation(warm[:], xc_pt[:], Exp)
            neg_xc = cpool.tile([128, NT], F32)
            nc.vector.tensor_scalar_mul(neg_xc[:], xc_pt[:], -1.0)

            # weights off the critical path: issue from the idle PE/gpsimd
            # queues so they don't serialize behind the xt/xc/r DMAs on SP
            wa = cpool.tile([C * KW + 1, C], F32)
            nc.gpsimd.dma_start(out=wa[:], in_=wconv[:])
            wl = cpool.tile([128, OUT_C], F32)
            nc.gpsimd.dma_start(out=wl[:], in_=wlin[:])
            blb = cpool.tile([128, OUT_C], F32)
            nc.gpsimd.dma_start(out=blb[:], in_=blin[0:1, :].partition_broadcast(128))

            # ---- conv im2col stack: row 0 = ones (bias), rows 1+16k+ci ----
            stack = cpool.tile([C * KW + 1, N_IN], F32)
            nc.vector.memset(stack[:, :], 0.0)
            pad = KW // 2
            for k in range(KW):
                lo = max(0, pad - k)
                hi = min(N_IN, N_IN + pad - k)
                eng = nc.gpsimd if k % 2 else nc.sync
                eng.dma_start(
                    out=stack[1 + C * k : 1 + C * (k + 1), lo:hi],
                    in_=r_in[:, lo + k - pad : hi + k - pad],
                )
            nc.vector.memset(stack[0:1, :], 1.0)

            # ---- conv matmuls: (81,128)^T @ (81,16) -> (128,16) per n-tile ----
            r_t = []
            for t in range(NT):
                cps = ppool.tile([128, C], F32, tag="smallps", bufs=2,
                                 name=f"cps{t}")
                nc.tensor.matmul(
                    cps[:],
                    lhsT=stack[:, t * 128 : (t + 1) * 128],
                    rhs=wa[:],
                    start=True,
                    stop=True,
                )
                # pad to 32 cols (zeros) so each z strip matmul writes a
                # full 32-partition group -> z4 has no undefined rows and
                # one whole-tile copy suffices
                rsb = cpool.tile([128, 2 * C], F32, name=f"rsb{t}")
                nc.vector.memset(rsb[:, C : 2 * C], 0.0)
                nc.vector.tensor_copy(out=rsb[:, 0:C], in_=cps[:])
                r_t.append(rsb)

            # ---- main pipeline over m-halves / n-tiles ----
            for mh in range(MH):
                if single_group:
                    a0 = groups[0][2]
                    z4 = ppool.tile([128, 512], F32, tag="z4", bufs=2,
                                    name=f"z4_{mh}")
                    for k in range(NT):
                        dsq = wpool.tile([128, 512], F32, tag="dsq", bufs=3,
                                         name=f"dsq{mh}_{k}")
                        if mh * NT + k in SQ_ON_ACT:
                            # fused (xt - xc_p)^2 on ACT via per-partition bias
                            nc.scalar.activation(dsq[:], xtb[mh][:], Square,
                                                 bias=neg_xc[:, k : k + 1])
                        else:
                            diff = wpool.tile([128, 512], F32, tag="diff",
                                              bufs=3, name=f"diff{mh}_{k}")
                            nc.vector.tensor_scalar(
                                diff[:], xtb[mh][:], xc_pt[:, k : k + 1], None,
                                op0=mybir.AluOpType.subtract,
                            )
                            nc.vector.tensor_mul(out=dsq[:], in0=diff[:],
                                                 in1=diff[:])
                        esb = wpool.tile([128, 512], F32, tag="esb", bufs=3,
                                         name=f"e{mh}_{k}")
                        nc.scalar.activation(esb[:], dsq[:], Exp,
                                             scale=-float(a0))
                        # one PE column strip per n-tile: the 4 matmuls run
                        # concurrently in the array
                        nc.tensor.matmul(
                            z4[32 * k : 32 * (k + 1), :],
                            lhsT=r_t[k][:],
                            rhs=esb[:],
                            start=True,
                            stop=True,
                            tile_position=(0, 32 * k),
                        )
                    # single whole-tile PSUM->SBUF copy; rows 32j+16..32j+31
                    # are computed zeros (padded lhsT), the linear matmul
                    # contracts over all 4 partials at no extra cost
                    zc = wpool.tile([128, 512], F32, tag="zc", bufs=2,
                                    name=f"zc{mh}")
                    nc.vector.tensor_copy(out=zc[:, :], in_=z4[:, :])
                    for mt in range(MT):
                        ops = ppool.tile([128, OUT_C], F32, tag="smallps",
                                         bufs=2, name=f"ops{mh}_{mt}")
                        nc.tensor.matmul(
                            ops[:],
                            lhsT=zc[:, mt * 128 : (mt + 1) * 128],
                            rhs=wl[:],
                            start=True,
                            stop=True,
                        )
                        osb = wpool.tile([128, OUT_C], F32, tag="osb", bufs=3,
                                         name=f"o{mh}_{mt}")
                        nc.vector.tensor_add(out=osb[:], in0=ops[:],
                                             in1=blb[:])
                        m0 = mh * 512 + mt * 128
                        eng = nc.gpsimd if mt % 2 else nc.sync
                        eng.dma_start(out=y_out[m0 : m0 + 128, :], in_=osb[:])
                    continue

                z_sb = wpool.tile([C, 512], F32, tag="zsb", bufs=2,
                                  name=f"z{mh}")
                if True:
                    # general path: per-group E maps, sequential PSUM accum
                    for gi, (c0, c1, ag) in enumerate(groups):
                        gsz = c1 - c0
                        zps = ppool.tile([gsz, 512], F32, tag="zps", bufs=2,
                                         name=f"zps{mh}_{gi}")
                        for k in range(NT):
                            diff = wpool.tile([128, 512], F32, tag="diff",
                                              bufs=3, name=f"df{mh}_{gi}_{k}")
                            nc.vector.tensor_scalar(
                                diff[:], xtb[mh][:], xc_pt[:, k : k + 1], None,
                                op0=mybir.AluOpType.subtract,
                            )
                            dsq = wpool.tile([128, 512], F32, tag="dsq",
                                             bufs=3, name=f"dq{mh}_{gi}_{k}")
                            nc.vector.tensor_mul(out=dsq[:], in0=diff[:],
                                                 in1=diff[:])
                            esb = wpool.tile([128, 512], F32, tag="esb",
                                             bufs=3, name=f"e{mh}_{gi}_{k}")
                            nc.scalar.activation(esb[:], dsq[:], Exp,
                                                 scale=-float(ag))
                            nc.tensor.matmul(
                                zps[:],
                                lhsT=r_t[k][:, c0:c1],
                                rhs=esb[:],
                                start=(k == 0),
                                stop=(k == NT - 1),
                            )
                        if c0 % 32 == 0:
                            nc.vector.tensor_copy(out=z_sb[c0:c1, :],
                                                  in_=zps[:])
                        else:
                            nc.sync.dma_start(out=z_sb[c0:c1, :], in_=zps[:])

                # ---- final linear; lin_b folded into the PSUM->SBUF copy ----
                for mt in range(MT):
                    ops = ppool.tile([128, OUT_C], F32, tag="smallps", bufs=2,
                                     name=f"ops{mh}_{mt}")
                    nc.tensor.matmul(
                        ops[:],
                        lhsT=z_sb[:, mt * 128 : (mt + 1) * 128],
                        rhs=wl[0:C, :],
                        start=True,
                        stop=True,
                    )
                    osb = wpool.tile([128, OUT_C], F32, tag="osb", bufs=3,
                                     name=f"o{mh}_{mt}")
                    nc.vector.tensor_add(out=osb[:], in0=ops[:], in1=blb[:])
                    m0 = mh * 512 + mt * 128
                    nc.sync.dma_start(out=y_out[m0 : m0 + 128, :], in_=osb[:])

    _split_multi_waits(nc)
    return nc


_cache = {}


def _get_nc(groups):
    key = tuple((c0, c1, np.float32(a).tobytes()) for c0, c1, a in groups)
    if key not in _cache:
        _cache[key] = _build(groups)
    return _cache[key]


def _prepare(r, x_context, y_context, x_target, conv_w, conv_b, sigma, lin_w,
             lin_b):
    r = np.asarray(r, np.float32)
    x_context = np.asarray(x_context, np.float32)
    x_target = np.asarray(x_target, np.float32)
    conv_w = np.asarray(conv_w, np.float32)
    conv_b = np.asarray(conv_b, np.float32)
    sigma = np.asarray(sigma, np.float32)
    lin_w = np.asarray(lin_w, np.float32)
    lin_b = np.asarray(lin_b, np.float32)

    # Channels sharing a length scale share one RBF map: sort channels by a,
    # group runs of equal values (uniform init sigma -> a single group).
    scales = np.exp(sigma.astype(np.float64))
    a = 0.5 / scales**2
    perm = np.argsort(a, kind="stable")
    a_s = a[perm]
    groups = []
    c0 = 0
    for c in range(1, C + 1):
        if c == C or a_s[c] != a_s[c0]:
            groups.append((c0, c, float(a_s[c0])))
            c0 = c
    groups = tuple(groups)

    # Repack weights (channel-permuted; conv bias row first, matching the
    # im2col ones row at partition 0).
    w_aug = np.concatenate(
        [conv_b[None, :], conv_w.transpose(2, 1, 0).reshape(C * KW, C)], axis=0
    )[:, perm]
    w_aug = np.ascontiguousarray(w_aug, np.float32)
    lin_w_t = lin_w.T[perm]
    lin128 = np.zeros((128, OUT_C), np.float32)
    for j in range(4):
        lin128[32 * j : 32 * j + C] = lin_w_t
    lin_b_row = np.ascontiguousarray(lin_b[None, :], np.float32)

    in_maps = [
        {
            "r": np.ascontiguousarray(r[b]),
            "xc": np.ascontiguousarray(x_context[b].reshape(1, N_IN)),
            "xt": np.ascontiguousarray(x_target[b].reshape(1, N_OUT)),
            "w_aug": w_aug,
            "lin128": lin128,
            "lin_b": lin_b_row,
        }
        for b in range(B)
    ]
    return groups, in_maps


def kernel(**inputs):
    groups, in_maps = _prepare(**inputs)
    nc = _get_nc(groups)
    res = run_bass_kernel_spmd(nc, in_maps, list(range(N_CORES)))
    return np.stack([res.results[b]["y"] for b in range(B)], axis=0)



# revision 6
# speedup vs baseline: 1.4016x; 1.4016x over previous
"""ConvDecoder Bass kernel for Trainium2, SPMD over 8 NeuronCores.

Math (per batch element b, one per core):
    r_conv = Conv1d(r, conv_w, SAME) + conv_b            # (C, N_IN)
    d[n,m] = (xc[n] - xt[m])^2                           # (N_IN, N_OUT)
    E_c    = exp(-a_c * d),  a_c = 0.5 / exp(sigma_c)^2
    z[m,c] = sum_n r_conv[c,n] * E_c[n,m]
    out    = z @ lin_w.T + lin_b                         # (N_OUT, OUT_C)

Per-core structure (v3):
  - conv as im2col matmul; the im2col stack (ones row for the bias + 5
    shifted copies of r) is built on HOST and arrives as ONE bf16 DMA.
    4 matmuls (one per n-tile of 128) write 16-col slices of one PSUM
    tile; a single DVE copy yields r_t (128n, 16c per tile) in bf16.
  - d^2 is computed ON THE PE as a rank-7 bf16 matmul:
        d2[n,m] = u^2 - 2uv + v^2
    with u = xc, v = xt split hi/lo into bf16 (u=uh+ul etc, squares
    pre-split on host) so every product is exactly representable:
      lhsT rows [s_uh, s_ul, -2uh, -2ul, -2uh, 1, 1, 0]   (8, 512)
      rhs  rows [1,    1,    vh,   vh,   vl, svh, svl, 0] (8, 1024)
    -> d2 lands in PSUM (128, 1024) per n-tile; worst-case error
    ~2e-4 absolute => ~1% in exp(-50 d2), far under tolerance.
    This removes the 512KB xt partition-broadcast DMA and all DVE
    diff/square work of the previous version.
  - ACT exp(scale=-a) reads d2 straight from PSUM, writes bf16 E to
    SBUF. ACT is the serial bottleneck (~(N+352)/1.2 ns per chunk);
    a warm exp on a memset tile triggers the ~2.7us table load at t~0.
  - z[c,m] accumulates over the 4 n-tiles into a (16, 512) PSUM tile
    per m-half as each E chunk appears (bf16 matmuls, 1-pass).
  - final linear: ones row appended to z copy (zc row 16) and lin_b as
    row 16 of the weights; 4 matmuls per m-half (contract 17) write
    (128m, 32o) PSUM, drained to SBUF by DVE/ACT alternately, stored
    with 8 contiguous 16KB DMAs spread over 3 queues.
"""

import numpy as np
import ml_dtypes

import concourse.bass as bass
import concourse.mybir as mybir
from concourse.tile import TileContext
from concourse.bass_utils import run_bass_kernel_spmd

F32 = mybir.dt.float32
BF16 = mybir.dt.bfloat16
BF = ml_dtypes.bfloat16

B, N_IN, N_OUT, C, OUT_C, KW = 8, 512, 1024, 16, 32, 5
N_CORES = 8
NT = N_IN // 128   # n tiles (4)
MH = N_OUT // 512  # m halves (2)
MT = 512 // 128    # m tiles per half (4)
ROWS = C * KW + 1  # im2col rows (81)


# --- walrus workaround -----------------------------------------------------
# This container's walrus accepts at most ONE semaphore wait per TPB
# instruction, but Tile's scheduler attaches several (joins + tail drain).
# Hoist all but the last wait of each instruction onto fresh wait-only
# EventSemaphore instructions inserted right before it on the same engine.
_ws_ctr = [0]


def _split_multi_waits(nc):
    for fn in nc.m.functions:
        for blk in fn.blocks:
            insts = blk.instructions
            if not any(
                ins.sync_info and len(ins.sync_info.on_wait) > 1 for ins in insts
            ):
                continue
            out = []
            for ins in insts:
                si = ins.sync_info
                waits = list(si.on_wait) if si else []
                if len(waits) > 1:
                    for w in waits[:-1]:
                        _ws_ctr[0] += 1
                        ev = mybir.InstEventSemaphore(
                            name=f"waitsplit_{_ws_ctr[0]}", ins=[], outs=[]
                        )
                        ev.engine = ins.engine
                        ev.sync_info = mybir.SyncInfo(on_wait=[w], on_update=[])
                        nc.register_instruction(ev)
                        out.append(ev)
                    ins.sync_info = mybir.SyncInfo(
                        on_wait=[waits[-1]], on_update=list(si.on_update)
                    )
                out.append(ins)
            insts[:] = out


# --- kernel build ----------------------------------------------------------
def _build(groups):
    """groups: tuple of (c0, c1, a) with contiguous channel ranges."""
    nc = bass.Bass()
    stack_d = nc.dram_tensor("stack", [ROWS, N_IN], BF16, kind="ExternalInput")
    uv_d = nc.dram_tensor("uv", [8, N_IN + N_OUT], BF16, kind="ExternalInput")
    wa_d = nc.dram_tensor("wa", [ROWS, C], BF16, kind="ExternalInput")
    wl_d = nc.dram_tensor("wl", [C + 1, OUT_C], BF16, kind="ExternalInput")
    y_d = nc.dram_tensor("y", [N_OUT, OUT_C], F32, kind="ExternalOutput")

    Exp = mybir.ActivationFunctionType.Exp
    single = len(groups) == 1

    with TileContext(nc) as tc:
        with (
            tc.tile_pool(name="const", bufs=1) as cpool,
            tc.tile_pool(name="work", bufs=1) as wpool,
            tc.tile_pool(name="psum", bufs=1, space="PSUM") as ppool,
        ):
            # --- warm exp: trigger the ACT table load at t~0 (no DMA dep) --
            wsrc = cpool.tile([8, 16], BF16)
            nc.vector.memset(wsrc[:], 0.0)
            wact = cpool.tile([8, 16], F32)
            nc.scalar.activation(wact[:], wsrc[:], Exp)

            # --- input DMAs: two queues, two issues each -------------------
            uvsb = cpool.tile([8, N_IN + N_OUT], BF16)
            nc.gpsimd.dma_start(out=uvsb[:], in_=uv_d[:])
            stack = cpool.tile([ROWS, N_IN], BF16)
            nc.sync.dma_start(out=stack[:], in_=stack_d[:])
            wa = cpool.tile([ROWS, C], BF16)
            nc.gpsimd.dma_start(out=wa[:], in_=wa_d[:])
            wl = cpool.tile([C + 1, OUT_C], BF16)
            nc.sync.dma_start(out=wl[:], in_=wl_d[:])

            uL = uvsb[:, 0:N_IN]            # (8, 512)  d2 lhsT rows
            vR = uvsb[:, N_IN:N_IN + N_OUT]  # (8, 1024) d2 rhs rows

            # --- d2 matmuls + exp per n-tile -------------------------------
            dsq = [
                ppool.tile([128, N_OUT], F32, tag="dsq", bufs=2,
                           name=f"dsq{k}")
                for k in range(NT)
            ]
            esb = {}
            for k in range(NT):
                for mh in range(MH):
                    nc.tensor.matmul(
                        dsq[k][:, mh * 512:(mh + 1) * 512],
                        lhsT=uL[:, k * 128:(k + 1) * 128],
                        rhs=vR[:, mh * 512:(mh + 1) * 512],
                        start=True,
                        stop=True,
                    )
                for gi, (c0, c1, ag) in enumerate(groups):
                    e = wpool.tile([128, N_OUT], BF16, tag="esb",
                                   bufs=5 if not single else NT,
                                   name=f"e{k}_{gi}")
                    nc.scalar.activation(e[:], dsq[k][:], Exp, scale=-float(ag))
                    esb[(k, gi)] = e

            # --- conv im2col matmuls --------------------------------------
            cps = ppool.tile([128, 4 * C], F32, tag="small", bufs=2)
            for k in range(NT):
                nc.tensor.matmul(
                    cps[:, k * C:(k + 1) * C],
                    lhsT=stack[:, k * 128:(k + 1) * 128],
                    rhs=wa[:],
                    start=True,
                    stop=True,
                )
            rsb = cpool.tile([128, 4 * C], BF16)
            nc.vector.tensor_copy(out=rsb[:], in_=cps[:])

            # --- z accumulation over n-tiles per m-half -------------------
            zps = {}
            for gi, (c0, c1, ag) in enumerate(groups):
                gsz = c1 - c0
                for mh in range(MH):
                    zp = ppool.tile([gsz, 512], F32, tag=f"zps{mh}", bufs=1,
                                    name=f"zps{mh}_{gi}")
                    zps[(gi, mh)] = zp
            for k in range(NT):
                for gi, (c0, c1, ag) in enumerate(groups):
                    for mh in range(MH):
                        nc.tensor.matmul(
                            zps[(gi, mh)][:],
                            lhsT=rsb[:, k * C + c0:k * C + c1],
                            rhs=esb[(k, gi)][:, mh * 512:(mh + 1) * 512],
                            start=(k == 0),
                            stop=(k == NT - 1),
                        )

            # --- zc: z in SBUF bf16 with a ones row for the bias ----------
            # memset ALL 17 rows to 1.0 (engine APs need base partition 0);
            # the z copies overwrite rows 0..15, leaving row 16 = ones,
            # which pairs with lin_b in wl's last row.
            zc = []
            for mh in range(MH):
                z = cpool.tile([C + 1, 512], BF16, name=f"zc{mh}")
                nc.vector.memset(z[:], 1.0)
                zc.append(z)
            for mh in range(MH):
                eng = nc.vector if mh == 0 else nc.scalar
                for gi, (c0, c1, ag) in enumerate(groups):
                    if eng is nc.vector:
                        eng.tensor_copy(out=zc[mh][c0:c1, :],
                                        in_=zps[(gi, mh)][:])
                    else:
                        eng.copy(out=zc[mh][c0:c1, :], in_=zps[(gi, mh)][:])

            # --- final linear + store -------------------------------------
            out_engs = [nc.sync, nc.gpsimd]
            for mh in range(MH):
                for mt in range(MT):
                    ops = ppool.tile([128, OUT_C], F32, tag="small", bufs=2,
                                     name=f"ops{mh}_{mt}")
                    nc.tensor.matmul(
                        ops[:],
                        lhsT=zc[mh][:, mt * 128:(mt + 1) * 128],
                        rhs=wl[:],
                        start=True,
                        stop=True,
                    )
                    osb = wpool.tile([128, OUT_C], F32, tag="osb", bufs=4,
                                     name=f"o{mh}_{mt}")
                    i = mh * MT + mt
                    if i % 2 == 0:
                        nc.vector.tensor_copy(out=osb[:], in_=ops[:])
                    else:
                        nc.scalar.copy(out=osb[:], in_=ops[:])
                    m0 = mh * 512 + mt * 128
                    out_engs[i % 2].dma_start(out=y_d[m0:m0 + 128, :],
                                              in_=osb[:])

    _split_multi_waits(nc)
    return nc


_cache = {}


def _get_nc(groups):
    key = tuple((c0, c1, np.float32(a).tobytes()) for c0, c1, a in groups)
    if key not in _cache:
        _cache[key] = _build(groups)
    return _cache[key]


def _hi_lo(x):
    """Split fp64 array into bf16 hi + bf16 lo with x ~ hi + lo."""
    hi = x.astype(BF)
    lo = (x - hi.astype(np.float64)).astype(BF)
    return hi, lo


def _prepare(r, x_context, y_context, x_target, conv_w, conv_b, sigma, lin_w,
             lin_b):
    r = np.asarray(r, np.float64)
    x_context = np.asarray(x_context, np.float64)
    x_target = np.asarray(x_target, np.float64)
    conv_w = np.asarray(conv_w, np.float64)
    conv_b = np.asarray(conv_b, np.float64)
    sigma = np.asarray(sigma, np.float64)
    lin_w = np.asarray(lin_w, np.float64)
    lin_b = np.asarray(lin_b, np.float64)

    # Channels sharing a length scale share one RBF map: sort channels by a,
    # group runs of equal values (uniform init sigma -> a single group).
    scales = np.exp(sigma)
    a = 0.5 / scales**2
    perm = np.argsort(a, kind="stable")
    a_s = a[perm]
    groups = []
    c0 = 0
    for c in range(1, C + 1):
        if c == C or a_s[c] != a_s[c0]:
            groups.append((c0, c, float(a_s[c0])))
            c0 = c
    groups = tuple(groups)

    # conv weights (channel-permuted), bias row first to pair with the
    # ones row of the im2col stack.
    w_aug = np.concatenate(
        [conv_b[None, :], conv_w.transpose(2, 1, 0).reshape(C * KW, C)], axis=0
    )[:, perm].astype(BF)
    # linear weights with lin_b as the last row (pairs with zc's ones row)
    wl = np.concatenate([lin_w.T[perm], lin_b[None, :]], axis=0).astype(BF)

    pad = KW // 2
    in_maps = []
    for b in range(B):
        # host im2col: ones row + 5 shifted copies of r (pure layout)
        stack = np.zeros((ROWS, N_IN), np.float64)
        stack[0] = 1.0
        rb = r[b]
        for k in range(KW):
            lo = max(0, pad - k)
            hi = min(N_IN, N_IN + pad - k)
            stack[1 + C * k:1 + C * (k + 1), lo:hi] = rb[:, lo + k - pad:hi + k - pad]

        # d2 factor rows: d2 = u^2 - 2uv + v^2 with exact bf16 products
        u = x_context[b, :, 0]
        v = x_target[b, :, 0]
        uh, ul = _hi_lo(u)
        vh, vl = _hi_lo(v)
        suh, sul = _hi_lo(u * u)
        svh, svl = _hi_lo(v * v)
        one_n = np.ones(N_IN, BF)
        one_m = np.ones(N_OUT, BF)
        zero_n = np.zeros(N_IN, BF)
        zero_m = np.zeros(N_OUT, BF)
        uL = np.stack([suh, sul,
                       (-2.0 * uh.astype(np.float64)).astype(BF),
                       (-2.0 * ul.astype(np.float64)).astype(BF),
                       (-2.0 * uh.astype(np.float64)).astype(BF),
                       one_n, one_n, zero_n])
        vR = np.stack([one_m, one_m, vh, vh, vl, svh, svl, zero_m])
        uv = np.concatenate([uL, vR], axis=1)

        in_maps.append({
            "stack": np.ascontiguousarray(stack.astype(BF)),
            "uv": np.ascontiguousarray(uv),
            "wa": np.ascontiguousarray(w_aug),
            "wl": np.ascontiguousarray(wl),
        })
    return groups, in_maps


def kernel(**inputs):
    groups, in_maps = _prepare(**inputs)
    nc = _get_nc(groups)
    res = run_bass_kernel_spmd(nc, in_maps, list(range(N_CORES)))
    return np.stack([res.results[b]["y"] for b in range(B)], axis=0)


# revision 9
# speedup vs baseline: 1.5152x; 1.0811x over previous
"""ConvDecoder Bass kernel for Trainium2, SPMD over 8 NeuronCores.

Math (per batch element b, one per core):
    r_conv = Conv1d(r, conv_w, SAME) + conv_b            # (C, N_IN)
    d[n,m] = (xc[n] - xt[m])^2                           # (N_IN, N_OUT)
    E_c    = exp(-a_c * d),  a_c = 0.5 / exp(sigma_c)^2
    z[m,c] = sum_n r_conv[c,n] * E_c[n,m]
    out    = z @ lin_w.T + lin_b                         # (N_OUT, OUT_C)

Per-core structure (v3):
  - conv as im2col matmul; the im2col stack (ones row for the bias + 5
    shifted copies of r) is built on HOST and arrives as ONE bf16 DMA.
    4 matmuls (one per n-tile of 128) write 16-col slices of one PSUM
    tile; a single DVE copy yields r_t (128n, 16c per tile) in bf16.
  - d^2 is computed ON THE PE as a rank-7 bf16 matmul:
        d2[n,m] = u^2 - 2uv + v^2
    with u = xc, v = xt split hi/lo into bf16 (u=uh+ul etc, squares
    pre-split on host) so every product is exactly representable:
      lhsT rows [s_uh, s_ul, -2uh, -2ul, -2uh, 1, 1, 0]   (8, 512)
      rhs  rows [1,    1,    vh,   vh,   vl, svh, svl, 0] (8, 1024)
    -> d2 lands in PSUM (128, 1024) per n-tile; worst-case error
    ~2e-4 absolute => ~1% in exp(-50 d2), far under tolerance.
    This removes the 512KB xt partition-broadcast DMA and all DVE
    diff/square work of the previous version.
  - ACT exp(scale=-a) reads d2 straight from PSUM, writes bf16 E to
    SBUF. ACT is the serial bottleneck (~(N+352)/1.2 ns per chunk);
    a warm exp on a memset tile triggers the ~2.7us table load at t~0.
  - z[c,m] accumulates over the 4 n-tiles into a (16, 512) PSUM tile
    per m-half as each E chunk appears (bf16 matmuls, 1-pass).
  - final linear: ones row appended to z copy (zc row 16) and lin_b as
    row 16 of the weights; 4 matmuls per m-half (contract 17) write
    (128m, 32o) PSUM, drained to SBUF by DVE/ACT alternately, stored
    with 8 contiguous 16KB DMAs spread over 3 queues.
"""

import numpy as np
import ml_dtypes

import concourse.bass as bass
import concourse.mybir as mybir
from concourse.tile import TileContext
from concourse.bass_utils import run_bass_kernel_spmd

F32 = mybir.dt.float32
BF16 = mybir.dt.bfloat16
BF = ml_dtypes.bfloat16

B, N_IN, N_OUT, C, OUT_C, KW = 8, 512, 1024, 16, 32, 5
N_CORES = 8
NT = N_IN // 128   # n tiles (4)
MH = N_OUT // 512  # m halves (2)
MT = 512 // 128    # m tiles per half (4)
ROWS = C * KW + 1  # im2col rows (81)


# --- walrus workaround -----------------------------------------------------
# This container's walrus accepts at most ONE semaphore wait per TPB
# instruction, but Tile's scheduler attaches several (joins + tail drain).
# Hoist all but the last wait of each instruction onto fresh wait-only
# EventSemaphore instructions inserted right before it on the same engine.
_ws_ctr = [0]


def _split_multi_waits(nc):
    for fn in nc.m.functions:
        for blk in fn.blocks:
            insts = blk.instructions
            if not any(
                ins.sync_info and len(ins.sync_info.on_wait) > 1 for ins in insts
            ):
                continue
            out = []
            for ins in insts:
                si = ins.sync_info
                waits = list(si.on_wait) if si else []
                if len(waits) > 1:
                    for w in waits[:-1]:
                        _ws_ctr[0] += 1
                        ev = mybir.InstEventSemaphore(
                            name=f"waitsplit_{_ws_ctr[0]}", ins=[], outs=[]
                        )
                        ev.engine = ins.engine
                        ev.sync_info = mybir.SyncInfo(on_wait=[w], on_update=[])
                        nc.register_instruction(ev)
                        out.append(ev)
                    ins.sync_info = mybir.SyncInfo(
                        on_wait=[waits[-1]], on_update=list(si.on_update)
                    )
                out.append(ins)
            insts[:] = out


# --- kernel build ----------------------------------------------------------
def _build(groups):
    """groups: tuple of (c0, c1, a) with contiguous channel ranges."""
    nc = bass.Bass()
    stack_d = nc.dram_tensor("stack", [ROWS, N_IN], BF16, kind="ExternalInput")
    uv_d = nc.dram_tensor("uv", [8, N_IN + N_OUT], BF16, kind="ExternalInput")
    wa_d = nc.dram_tensor("wa", [ROWS, C], BF16, kind="ExternalInput")
    wl_d = nc.dram_tensor("wl", [C + 1, OUT_C], BF16, kind="ExternalInput")
    y_d = nc.dram_tensor("y", [N_OUT, OUT_C], F32, kind="ExternalOutput")

    Exp = mybir.ActivationFunctionType.Exp
    single = len(groups) == 1

    with TileContext(nc) as tc:
        with (
            tc.tile_pool(name="const", bufs=1) as cpool,
            tc.tile_pool(name="work", bufs=1) as wpool,
            tc.tile_pool(name="psum", bufs=1, space="PSUM") as ppool,
        ):
            # --- warm exp: trigger the ACT table load at t~0 (no DMA dep) --
            wsrc = cpool.tile([8, 640], BF16)
            nc.vector.memset(wsrc[:], 0.0)
            wact = cpool.tile([8, 16], F32)
            nc.scalar.activation(wact[:], wsrc[:, 0:16], Exp)

            # --- input DMAs: two queues, two issues each -------------------
            # uv gates the d2 pipeline: first on the sync queue.
            uvsb = cpool.tile([8, N_IN + N_OUT], BF16)
            nc.sync.dma_start(out=uvsb[:], in_=uv_d[:])
            stack = cpool.tile([ROWS, N_IN], BF16)
            nc.gpsimd.dma_start(out=stack[:], in_=stack_d[:])
            wa = cpool.tile([ROWS, C], BF16)
            nc.gpsimd.dma_start(out=wa[:], in_=wa_d[:])
            wl = cpool.tile([C + 1, OUT_C], BF16)
            nc.sync.dma_start(out=wl[:], in_=wl_d[:])

            # --- PE warmups: run during the DMA-latency dead zone so the
            # HAM clock gate releases (1.2 -> 2.4 GHz) before real matmuls,
            # and the PE queue has work while input DMAs land.
            wps = ppool.tile([128, 512], F32, tag="small", bufs=2)
            for i in range(6):
                nc.tensor.matmul(
                    wps[:],
                    lhsT=wsrc[:, 0:128],
                    rhs=wsrc[:, 128:640],
                    start=True,
                    stop=True,
                )

            uL = uvsb[:, 0:N_IN]            # (8, 512)  d2 lhsT rows
            vR = uvsb[:, N_IN:N_IN + N_OUT]  # (8, 1024) d2 rhs rows

            # --- d2 matmuls + exp per n-tile -------------------------------
            dsq = [
                ppool.tile([128, N_OUT], F32, tag="dsq", bufs=2,
                           name=f"dsq{k}")
                for k in range(NT)
            ]
            esb = {}
            for k in range(NT):
                for mh in range(MH):
                    nc.tensor.matmul(
                        dsq[k][:, mh * 512:(mh + 1) * 512],
                        lhsT=uL[:, k * 128:(k + 1) * 128],
                        rhs=vR[:, mh * 512:(mh + 1) * 512],
                        start=True,
                        stop=True,
                    )
                for gi, (c0, c1, ag) in enumerate(groups):
                    e = wpool.tile([128, N_OUT], BF16, tag="esb",
                                   bufs=5 if not single else NT,
                                   name=f"e{k}_{gi}")
                    nc.scalar.activation(e[:], dsq[k][:], Exp, scale=-float(ag))
                    esb[(k, gi)] = e

            # --- conv im2col matmuls --------------------------------------
            cps = ppool.tile([128, 4 * C], F32, tag="small", bufs=2)
            for k in range(NT):
                nc.tensor.matmul(
                    cps[:, k * C:(k + 1) * C],
                    lhsT=stack[:, k * 128:(k + 1) * 128],
                    rhs=wa[:],
                    start=True,
                    stop=True,
                )
            rsb = cpool.tile([128, 4 * C], BF16)
            nc.vector.tensor_copy(out=rsb[:], in_=cps[:])

            # --- z accumulation over n-tiles per m-half -------------------
            zps = {}
            for gi, (c0, c1, ag) in enumerate(groups):
                gsz = c1 - c0
                for mh in range(MH):
                    zp = ppool.tile([gsz, 512], F32, tag=f"zps{mh}", bufs=1,
                                    name=f"zps{mh}_{gi}")
                    zps[(gi, mh)] = zp
            for k in range(NT):
                for gi, (c0, c1, ag) in enumerate(groups):
                    for mh in range(MH):
                        nc.tensor.matmul(
                            zps[(gi, mh)][:],
                            lhsT=rsb[:, k * C + c0:k * C + c1],
                            rhs=esb[(k, gi)][:, mh * 512:(mh + 1) * 512],
                            start=(k == 0),
                            stop=(k == NT - 1),
                        )

            # --- zc: z in SBUF bf16 with a ones row for the bias ----------
            # memset ALL 17 rows to 1.0 (engine APs need base partition 0);
            # the z copies overwrite rows 0..15, leaving row 16 = ones,
            # which pairs with lin_b in wl's last row.
            zc = []
            for mh in range(MH):
                z = cpool.tile([C + 1, 512], BF16, name=f"zc{mh}")
                nc.vector.memset(z[:], 1.0)
                zc.append(z)
            for mh in range(MH):
                eng = nc.vector if mh == 0 else nc.scalar
                for gi, (c0, c1, ag) in enumerate(groups):
                    if eng is nc.vector:
                        eng.tensor_copy(out=zc[mh][c0:c1, :],
                                        in_=zps[(gi, mh)][:])
                    else:
                        eng.copy(out=zc[mh][c0:c1, :], in_=zps[(gi, mh)][:])

            # --- final linear + store -------------------------------------
            # 4 col-slice matmuls into ONE psum bank per m-half, one drain,
            # one DMA per half (dst AP rearranged to the (m, o) layout).
            out_engs = [nc.sync, nc.gpsimd]
            for mh in range(MH):
                lps = ppool.tile([128, MT * OUT_C], F32, tag="small", bufs=2,
                                 name=f"lps{mh}")
                for mt in range(MT):
                    nc.tensor.matmul(
                        lps[:, mt * OUT_C:(mt + 1) * OUT_C],
                        lhsT=zc[mh][:, mt * 128:(mt + 1) * 128],
                        rhs=wl[:],
                        start=True,
                        stop=True,
                    )
                osb = wpool.tile([128, MT * OUT_C], F32, tag="osb", bufs=2,
                                 name=f"o{mh}")
                if mh == 0:
                    nc.vector.tensor_copy(out=osb[:], in_=lps[:])
                else:
                    nc.scalar.copy(out=osb[:], in_=lps[:])
                y_r = y_d[mh * 512:(mh + 1) * 512, :].rearrange(
                    "(t p) o -> p t o", p=128
                )
                out_engs[mh].dma_start(out=y_r, in_=osb[:])

    _split_multi_waits(nc)
    return nc


_cache = {}


def _get_nc(groups):
    key = tuple((c0, c1, np.float32(a).tobytes()) for c0, c1, a in groups)
    if key not in _cache:
        _cache[key] = _build(groups)
    return _cache[key]


def _hi_lo(x):
    """Split fp64 array into bf16 hi + bf16 lo with x ~ hi + lo."""
    hi = x.astype(BF)
    lo = (x - hi.astype(np.float64)).astype(BF)
    return hi, lo


def _prepare(r, x_context, y_context, x_target, conv_w, conv_b, sigma, lin_w,
             lin_b):
    r = np.asarray(r, np.float64)
    x_context = np.asarray(x_context, np.float64)
    x_target = np.asarray(x_target, np.float64)
    conv_w = np.asarray(conv_w, np.float64)
    conv_b = np.asarray(conv_b, np.float64)
    sigma = np.asarray(sigma, np.float64)
    lin_w = np.asarray(lin_w, np.float64)
    lin_b = np.asarray(lin_b, np.float64)

    # Channels sharing a length scale share one RBF map: sort channels by a,
    # group runs of equal values (uniform init sigma -> a single group).
    scales = np.exp(sigma)
    a = 0.5 / scales**2
    perm = np.argsort(a, kind="stable")
    a_s = a[perm]
    groups = []
    c0 = 0
    for c in range(1, C + 1):
        if c == C or a_s[c] != a_s[c0]:
            groups.append((c0, c, float(a_s[c0])))
            c0 = c
    groups = tuple(groups)

    # conv weights (channel-permuted), bias row first to pair with the
    # ones row of the im2col stack.
    w_aug = np.concatenate(
        [conv_b[None, :], conv_w.transpose(2, 1, 0).reshape(C * KW, C)], axis=0
    )[:, perm].astype(BF)
    # linear weights with lin_b as the last row (pairs with zc's ones row)
    wl = np.concatenate([lin_w.T[perm], lin_b[None, :]], axis=0).astype(BF)

    pad = KW // 2
    in_maps = []
    for b in range(B):
        # host im2col: ones row + 5 shifted copies of r (pure layout)
        stack = np.zeros((ROWS, N_IN), np.float64)
        stack[0] = 1.0
        rb = r[b]
        for k in range(KW):
            lo = max(0, pad - k)
            hi = min(N_IN, N_IN + pad - k)
            stack[1 + C * k:1 + C * (k + 1), lo:hi] = rb[:, lo + k - pad:hi + k - pad]

        # d2 factor rows: d2 = u^2 - 2uv + v^2 with exact bf16 products
        u = x_context[b, :, 0]
        v = x_target[b, :, 0]
        uh, ul = _hi_lo(u)
        vh, vl = _hi_lo(v)
        suh, sul = _hi_lo(u * u)
        svh, svl = _hi_lo(v * v)
        one_n = np.ones(N_IN, BF)
        one_m = np.ones(N_OUT, BF)
        zero_n = np.zeros(N_IN, BF)
        zero_m = np.zeros(N_OUT, BF)
        uL = np.stack([suh, sul,
                       (-2.0 * uh.astype(np.float64)).astype(BF),
                       (-2.0 * ul.astype(np.float64)).astype(BF),
                       (-2.0 * uh.astype(np.float64)).astype(BF),
                       one_n, one_n, zero_n])
        vR = np.stack([one_m, one_m, vh, vh, vl, svh, svl, zero_m])
        uv = np.concatenate([uL, vR], axis=1)

        in_maps.append({
            "stack": np.ascontiguousarray(stack.astype(BF)),
            "uv": np.ascontiguousarray(uv),
            "wa": np.ascontiguousarray(w_aug),
            "wl": np.ascontiguousarray(wl),
        })
    return groups, in_maps


def kernel(**inputs):
    groups, in_maps = _prepare(**inputs)
    nc = _get_nc(groups)
    res = run_bass_kernel_spmd(nc, in_maps, list(range(N_CORES)))
    return np.stack([res.results[b]["y"] for b in range(B)], axis=0)


# revision 13
# speedup vs baseline: 1.7426x; 1.1501x over previous
"""ConvDecoder Bass kernel for Trainium2, SPMD over 8 NeuronCores.

Math (per batch element b, one per core):
    r_conv = Conv1d(r, conv_w, SAME) + conv_b            # (C, N_IN)
    d[n,m] = (xc[n] - xt[m])^2                           # (N_IN, N_OUT)
    E_c    = exp(-a_c * d),  a_c = 0.5 / exp(sigma_c)^2
    z[m,c] = sum_n r_conv[c,n] * E_c[n,m]
    out    = z @ lin_w.T + lin_b                         # (N_OUT, OUT_C)

Per-core structure (v5):
  - conv as im2col matmul; the im2col stack (ones row for the bias + 5
    shifted copies of r) is built on HOST, column-permuted into sorted
    xc order, and arrives as ONE bf16 DMA.
  - d^2 is computed ON THE PE as a rank-7 bf16 matmul:
        d2[n,m] = u^2 - 2uv + v^2
    with u = xc, v = xt split hi/lo into bf16 so every product is
    exactly representable (error ~2e-4 absolute in d2).  The 4 n-tiles
    use row-tiled tile_position=(32k, 0) so their matmuls overlap in
    the PE array.  d2 lands in (128, 1024) PSUM; ACT exp(scale=-a)
    reads PSUM directly and writes bf16 E to SBUF.  ACT is the serial
    bottleneck; a warm exp on a memset tile pulls the ~2.7us table
    load to t~0, and warmup matmuls keep the PE busy through the
    input-DMA latency (HAM clock-gate release).
  - BANDING: xc and xt are sorted on host, so distant (n-tile, m-half)
    blocks have exp(-a d^2) ~ 0 and are skipped entirely (d2 matmul,
    exp, and z matmul).  The block mask is computed per run from the
    actual data (union over batches) and baked into the compiled
    kernel (cache key).  Typically 6 of 8 blocks survive, and the
    m-half-0 tail (z copy, linear, drain) overlaps the last ACT chunk.
  - z[c,m] accumulates over active n-tiles into one (48, 512) PSUM
    bank: m-half 0 in partitions 0:16 (col group 0), m-half 1 in
    32:48 (col group 1) so the two matmuls per n-tile run concurrently.
  - final linear: ones row 16 of the z copy pairs with lin_b in the
    last row of the weights; 4 col-slice matmuls per m-half into one
    PSUM bank, one drain per half (DVE / ACT), ONE contiguous 128KB
    output DMA in device layout; the host inverse-gathers the rows
    (sorted -> original xt order) for free.
"""

import numpy as np
import ml_dtypes

import concourse.bass as bass
import concourse.mybir as mybir
from concourse.tile import TileContext
from concourse.bass_utils import run_bass_kernel_spmd

F32 = mybir.dt.float32
BF16 = mybir.dt.bfloat16
BF = ml_dtypes.bfloat16

B, N_IN, N_OUT, C, OUT_C, KW = 8, 512, 1024, 16, 32, 5
N_CORES = 8
NT = N_IN // 128   # n tiles (4)
MH = N_OUT // 512  # m halves (2)
MT = 512 // 128    # m tiles per half (4)
ROWS = C * KW + 1  # im2col rows (81)
BAND_T = 16.0      # skip a block when a * gap^2 > BAND_T (exp < 1.2e-7)


# --- walrus workaround -----------------------------------------------------
# This container's walrus accepts at most ONE semaphore wait per TPB
# instruction, but Tile's scheduler attaches several (joins + tail drain).
# Hoist all but the last wait of each instruction onto fresh wait-only
# EventSemaphore instructions inserted right before it on the same engine.
_ws_ctr = [0]


def _split_multi_waits(nc):
    for fn in nc.m.functions:
        for blk in fn.blocks:
            insts = blk.instructions
            if not any(
                ins.sync_info and len(ins.sync_info.on_wait) > 1 for ins in insts
            ):
                continue
            out = []
            for ins in insts:
                si = ins.sync_info
                waits = list(si.on_wait) if si else []
                if len(waits) > 1:
                    for w in waits[:-1]:
                        _ws_ctr[0] += 1
                        ev = mybir.InstEventSemaphore(
                            name=f"waitsplit_{_ws_ctr[0]}", ins=[], outs=[]
                        )
                        ev.engine = ins.engine
                        ev.sync_info = mybir.SyncInfo(on_wait=[w], on_update=[])
                        nc.register_instruction(ev)
                        out.append(ev)
                    ins.sync_info = mybir.SyncInfo(
                        on_wait=[waits[-1]], on_update=list(si.on_update)
                    )
                out.append(ins)
            insts[:] = out


# --- kernel build ----------------------------------------------------------
def _build(key):
    """key: (groups, active) with groups ((c0, c1, a), ...) and active a
    tuple over n-tiles of frozensets of active m-halves."""
    groups, active = key
    single = len(groups) == 1
    nc = bass.Bass()
    stack_d = nc.dram_tensor("stack", [ROWS, N_IN], BF16, kind="ExternalInput")
    uv_d = nc.dram_tensor("uv", [8, N_IN + N_OUT], BF16, kind="ExternalInput")
    wa_d = nc.dram_tensor("wa", [ROWS, C], BF16, kind="ExternalInput")
    wl_d = nc.dram_tensor("wl", [C + 1, OUT_C], BF16, kind="ExternalInput")
    y_d = nc.dram_tensor("y", [N_OUT, OUT_C], F32, kind="ExternalOutput")

    Exp = mybir.ActivationFunctionType.Exp
    # active m-col range per n-tile (contiguous since halves are sorted)
    lo_hi = []
    for k in range(NT):
        mhs = sorted(active[k])
        lo_hi.append((mhs[0] * 512, (mhs[-1] + 1) * 512) if mhs else None)
    # active n-tiles per m-half
    ks_of = [
        [k for k in range(NT) if mh in active[k]] for mh in range(MH)
    ]

    with TileContext(nc) as tc:
        with (
            tc.tile_pool(name="const", bufs=1) as cpool,
            tc.tile_pool(name="work", bufs=1) as wpool,
            tc.tile_pool(name="psum", bufs=1, space="PSUM") as ppool,
        ):
            # --- warm exp: trigger the ACT table load at t~0 (no DMA dep) --
            wsrc = cpool.tile([8, 640], BF16)
            nc.vector.memset(wsrc[:], 0.0)
            wact = cpool.tile([8, 16], F32)
            nc.scalar.activation(wact[:], wsrc[:, 0:16], Exp)

            # --- input DMAs --------------------------------------------
            # uv gates the d2 pipeline: first on the sync queue.  Row-tiled
            # d2 matmuls need the operands at partition base 32k, so the 8
            # uv rows are replicated to 4 partition offsets (the k=0 copy
            # is the only one on the critical path).
            uvsb = cpool.tile([104, N_IN + N_OUT], BF16)
            uvq = [nc.sync, nc.gpsimd, nc.sync, nc.gpsimd]
            for k in range(NT):
                uvq[k].dma_start(out=uvsb[32 * k:32 * k + 8, :], in_=uv_d[:])
            stack = cpool.tile([ROWS, N_IN], BF16)
            nc.gpsimd.dma_start(out=stack[:], in_=stack_d[:])
            wa = cpool.tile([ROWS, C], BF16)
            nc.gpsimd.dma_start(out=wa[:], in_=wa_d[:])
            wl = cpool.tile([C + 1, OUT_C], BF16)
            nc.sync.dma_start(out=wl[:], in_=wl_d[:])

            # --- PE warmups: cover the DMA-latency dead zone so the HAM
            # clock gate can release and the queue isn't idle.
            wps = ppool.tile([128, 512], F32, tag="small", bufs=2)
            for i in range(4):
                nc.tensor.matmul(
                    wps[:],
                    lhsT=wsrc[:, 0:128],
                    rhs=wsrc[:, 128:640],
                    start=True,
                    stop=True,
                )

            def uL(k):   # (8, 128) d2 lhsT rows for n-tile k, at base 32k
                return uvsb[32 * k:32 * k + 8, k * 128:(k + 1) * 128]

            def vR(k, mh):  # (8, 512) d2 rhs rows at base 32k
                return uvsb[32 * k:32 * k + 8,
                            N_IN + mh * 512:N_IN + (mh + 1) * 512]

            # --- d2 matmuls (row-tiled) + exp per n-tile -------------------
            dsq = [
                ppool.tile([128, N_OUT], F32, tag="dsq", bufs=2,
                           name=f"dsq{k}")
                for k in range(NT)
            ]
            esb = {}
            for k in range(NT):
                if lo_hi[k] is None:
                    continue
                lo, hi = lo_hi[k]
                for mh in sorted(active[k]):
                    nc.tensor.matmul(
                        dsq[k][:, mh * 512:(mh + 1) * 512],
                        lhsT=uL(k),
                        rhs=vR(k, mh),
                        start=True,
                        stop=True,
                        tile_position=(32 * k, 0),
                    )
                for gi, (c0, c1, ag) in enumerate(groups):
                    e = wpool.tile([128, N_OUT], BF16, tag="esb",
                                   bufs=NT if single else 6,
                                   name=f"e{k}_{gi}")
                    nc.scalar.activation(e[:, lo:hi], dsq[k][:, lo:hi], Exp,
                                         scale=-float(ag))
                    esb[(k, gi)] = e

            # --- conv im2col matmuls --------------------------------------
            cps = ppool.tile([128, 4 * C], F32, tag="small", bufs=2)
            for k in range(NT):
                nc.tensor.matmul(
                    cps[:, k * C:(k + 1) * C],
                    lhsT=stack[:, k * 128:(k + 1) * 128],
                    rhs=wa[:],
                    start=True,
                    stop=True,
                )
            rsb = cpool.tile([128, 4 * C], BF16)
            nc.vector.tensor_copy(out=rsb[:], in_=cps[:])

            # --- z accumulation over active n-tiles per m-half ------------
            if single:
                # one PSUM bank: mh0 in partitions 0:16 (col group 0),
                # mh1 in 32:48 (col group 1) -> concurrent matmuls
                zps = ppool.tile([48, 512], F32, tag="zps", bufs=1)
                zsl = [zps[0:16, :], zps[32:48, :]]
                for k in range(NT):
                    for mh in range(MH):
                        if mh not in active[k]:
                            continue
                        nc.tensor.matmul(
                            zsl[mh],
                            lhsT=rsb[:, k * C:k * C + 16],
                            rhs=esb[(k, 0)][:, mh * 512:(mh + 1) * 512],
                            start=(k == ks_of[mh][0]),
                            stop=(k == ks_of[mh][-1]),
                            tile_position=(0, 32 * mh),
                        )
            else:
                zps_g = {}
                for gi, (c0, c1, ag) in enumerate(groups):
                    for mh in range(MH):
                        zps_g[(gi, mh)] = ppool.tile(
                            [c1 - c0, 512], F32, tag=f"zps{mh}", bufs=1,
                            name=f"zps{mh}_{gi}")
                for k in range(NT):
                    for gi, (c0, c1, ag) in enumerate(groups):
                        for mh in range(MH):
                            if mh not in active[k]:
                                continue
                            nc.tensor.matmul(
                                zps_g[(gi, mh)][:],
                                lhsT=rsb[:, k * C + c0:k * C + c1],
                                rhs=esb[(k, gi)][:, mh * 512:(mh + 1) * 512],
                                start=(k == ks_of[mh][0]),
                                stop=(k == ks_of[mh][-1]),
                            )

            # --- zc: z in SBUF bf16 with a ones row for the bias ----------
            # memset ALL 17 rows to 1.0 (engine APs need base partition 0);
            # the z copies overwrite rows 0..15, leaving row 16 = ones,
            # which pairs with lin_b in wl's last row.
            zc = []
            for mh in range(MH):
                z = cpool.tile([C + 1, 512], BF16, name=f"zc{mh}")
                nc.vector.memset(z[:], 1.0)
                zc.append(z)
            zeng = [nc.vector, nc.scalar]
            for mh in range(MH):
                if single:
                    if mh == 0:
                        nc.vector.tensor_copy(out=zc[0][0:16, :], in_=zsl[0])
                    else:
                        nc.scalar.copy(out=zc[1][0:16, :], in_=zsl[1])
                else:
                    # general path: stage each group at partition 0, then
                    # SBUF->SBUF DMA into its channel rows (engines cannot
                    # address partition bases outside {0,32,64,96}).
                    for gi, (c0, c1, ag) in enumerate(groups):
                        stg = wpool.tile([c1 - c0, 512], BF16, tag="zstg",
                                         bufs=2, name=f"zstg{mh}_{gi}")
                        nc.vector.tensor_copy(out=stg[:],
                                              in_=zps_g[(gi, mh)][:])
                        nc.gpsimd.dma_start(out=zc[mh][c0:c1, :], in_=stg[:])

            # --- final linear + one contiguous store ----------------------
            # osb[p, mh*128 + mt*32 + o] -> y row 8p + 4mh + mt (device
            # layout; host inverse-gathers back to original xt order).
            osb = wpool.tile([128, 2 * MT * OUT_C], F32, tag="osb", bufs=1)
            for mh in range(MH):
                lps = ppool.tile([128, MT * OUT_C], F32, tag="small", bufs=2,
                                 name=f"lps{mh}")
                for mt in range(MT):
                    nc.tensor.matmul(
                        lps[:, mt * OUT_C:(mt + 1) * OUT_C],
                        lhsT=zc[mh][:, mt * 128:(mt + 1) * 128],
                        rhs=wl[:],
                        start=True,
                        stop=True,
                    )
                dst = osb[:, mh * 128:(mh + 1) * 128]
                if mh == 0:
                    nc.vector.tensor_copy(out=dst, in_=lps[:])
                else:
                    nc.scalar.copy(out=dst, in_=lps[:])
            y_r = y_d.rearrange("(p j) o -> p j o", p=128)
            nc.sync.dma_start(out=y_r, in_=osb[:])

    _split_multi_waits(nc)
    return nc


_cache = {}


def _get_nc(key):
    if key not in _cache:
        _cache[key] = _build(key)
    return _cache[key]


def _hi_lo(x):
    """Split fp64 array into bf16 hi + bf16 lo with x ~ hi + lo."""
    hi = x.astype(BF)
    lo = (x - hi.astype(np.float64)).astype(BF)
    return hi, lo


def _prepare(r, x_context, y_context, x_target, conv_w, conv_b, sigma, lin_w,
             lin_b):
    r = np.asarray(r, np.float64)
    x_context = np.asarray(x_context, np.float64)
    x_target = np.asarray(x_target, np.float64)
    conv_w = np.asarray(conv_w, np.float64)
    conv_b = np.asarray(conv_b, np.float64)
    sigma = np.asarray(sigma, np.float64)
    lin_w = np.asarray(lin_w, np.float64)
    lin_b = np.asarray(lin_b, np.float64)

    # Channels sharing a length scale share one RBF map: sort channels by a,
    # group runs of equal values (uniform init sigma -> a single group).
    scales = np.exp(sigma)
    a = 0.5 / scales**2
    perm = np.argsort(a, kind="stable")
    a_s = a[perm]
    groups = []
    c0 = 0
    for c in range(1, C + 1):
        if c == C or a_s[c] != a_s[c0]:
            groups.append((c0, c, float(a_s[c0])))
            c0 = c
    groups = tuple(groups)
    a_min = a_s[0]

    # conv weights (channel-permuted), bias row first to pair with the
    # ones row of the im2col stack.
    w_aug = np.concatenate(
        [conv_b[None, :], conv_w.transpose(2, 1, 0).reshape(C * KW, C)], axis=0
    )[:, perm].astype(BF)
    # linear weights with lin_b as the last row (pairs with zc's ones row)
    wl = np.concatenate([lin_w.T[perm], lin_b[None, :]], axis=0).astype(BF)

    pad = KW // 2
    in_maps = []
    act_sets = [set() for _ in range(NT)]
    gathers = []
    for b in range(B):
        u_raw = x_context[b, :, 0]
        v_raw = x_target[b, :, 0]
        u_idx = np.argsort(u_raw, kind="stable")
        v_idx = np.argsort(v_raw, kind="stable")
        u = u_raw[u_idx]
        v = v_raw[v_idx]

        # banding mask: block (n-tile k, m-half mh) active iff the sorted
        # value ranges come within gap, a*gap^2 <= BAND_T
        for k in range(NT):
            ulo, uhi = u[k * 128], u[(k + 1) * 128 - 1]
            for mh in range(MH):
                vlo, vhi = v[mh * 512], v[(mh + 1) * 512 - 1]
                gap = max(0.0, max(ulo - vhi, vlo - uhi))
                if a_min * gap * gap <= BAND_T:
                    act_sets[k].add(mh)

        # host im2col: ones row + 5 shifted copies of r (pure layout),
        # then permute columns into sorted-xc order
        stack = np.zeros((ROWS, N_IN), np.float64)
        stack[0] = 1.0
        rb = r[b]
        for k in range(KW):
            lo = max(0, pad - k)
            hi = min(N_IN, N_IN + pad - k)
            stack[1 + C * k:1 + C * (k + 1), lo:hi] = rb[:, lo + k - pad:hi + k - pad]
        stack = stack[:, u_idx]

        # d2 factor rows: d2 = u^2 - 2uv + v^2 with exact bf16 products
        uh, ul = _hi_lo(u)
        vh, vl = _hi_lo(v)
        suh, sul = _hi_lo(u * u)
        svh, svl = _hi_lo(v * v)
        one_n = np.ones(N_IN, BF)
        one_m = np.ones(N_OUT, BF)
        zero_n = np.zeros(N_IN, BF)
        zero_m = np.zeros(N_OUT, BF)
        uL = np.stack([suh, sul,
                       (-2.0 * uh.astype(np.float64)).astype(BF),
                       (-2.0 * ul.astype(np.float64)).astype(BF),
                       (-2.0 * uh.astype(np.float64)).astype(BF),
                       one_n, one_n, zero_n])
        vR = np.stack([one_m, one_m, vh, vh, vl, svh, svl, zero_m])
        uv = np.concatenate([uL, vR], axis=1)

        in_maps.append({
            "stack": np.ascontiguousarray(stack.astype(BF)),
            "uv": np.ascontiguousarray(uv),
            "wa": np.ascontiguousarray(w_aug),
            "wl": np.ascontiguousarray(wl),
        })

        # device row for sorted rank rk (= mh*512 + mt*128 + p) is
        # 8p + 4mh + mt; output row v_idx[rk] gets device row devrow(rk)
        rk = np.arange(N_OUT)
        mh_, mt_, p_ = rk // 512, (rk // 128) % 4, rk % 128
        devrow = 8 * p_ + 4 * mh_ + mt_
        inv = np.empty(N_OUT, np.int64)
        inv[v_idx] = devrow
        gathers.append(inv)

    active = tuple(frozenset(s) for s in act_sets)
    key = (groups, active)
    return key, in_maps, gathers


def _assemble(res, gathers):
    return np.stack(
        [res.results[b]["y"][gathers[b]] for b in range(B)], axis=0
    )


def kernel(**inputs):
    key, in_maps, gathers = _prepare(**inputs)
    nc = _get_nc(key)
    res = run_bass_kernel_spmd(nc, in_maps, list(range(N_CORES)))
    return _assemble(res, gathers)


# revision 21
# speedup vs baseline: 1.8045x; 1.0355x over previous
"""ConvDecoder Bass kernel for Trainium2, SPMD over 8 NeuronCores.

Math (per batch element b, one per core):
    r_conv = Conv1d(r, conv_w, SAME) + conv_b            # (C, N_IN)
    d[n,m] = (xc[n] - xt[m])^2                           # (N_IN, N_OUT)
    E_c    = exp(-a_c * d),  a_c = 0.5 / exp(sigma_c)^2
    z[m,c] = sum_n r_conv[c,n] * E_c[n,m]
    out    = z @ lin_w.T + lin_b                         # (N_OUT, OUT_C)

Per-core structure (v5):
  - conv as im2col matmul; the im2col stack (ones row for the bias + 5
    shifted copies of r) is built on HOST, column-permuted into sorted
    xc order, and arrives as ONE bf16 DMA.
  - d^2 is computed ON THE PE as a rank-7 bf16 matmul:
        d2[n,m] = u^2 - 2uv + v^2
    with u = xc, v = xt split hi/lo into bf16 so every product is
    exactly representable (error ~2e-4 absolute in d2).  The 4 n-tiles
    use row-tiled tile_position=(32k, 0) so their matmuls overlap in
    the PE array.  d2 lands in (128, 1024) PSUM; ACT exp(scale=-a)
    reads PSUM directly and writes bf16 E to SBUF.  ACT is the serial
    bottleneck; a warm exp on a memset tile pulls the ~2.7us table
    load to t~0, and warmup matmuls keep the PE busy through the
    input-DMA latency (HAM clock-gate release).
  - BANDING: xc and xt are sorted on host, so distant (n-tile, m-half)
    blocks have exp(-a d^2) ~ 0 and are skipped entirely (d2 matmul,
    exp, and z matmul).  The block mask is computed per run from the
    actual data (union over batches) and baked into the compiled
    kernel (cache key).  Typically 6 of 8 blocks survive, and the
    m-half-0 tail (z copy, linear, drain) overlaps the last ACT chunk.
  - z[c,m] accumulates over active n-tiles into one (48, 512) PSUM
    bank: m-half 0 in partitions 0:16 (col group 0), m-half 1 in
    32:48 (col group 1) so the two matmuls per n-tile run concurrently.
  - final linear: ones row 16 of the z copy pairs with lin_b in the
    last row of the weights; 4 col-slice matmuls per m-half into one
    PSUM bank, one drain per half (DVE / ACT), ONE contiguous 128KB
    output DMA in device layout; the host inverse-gathers the rows
    (sorted -> original xt order) for free.
"""

import numpy as np
import ml_dtypes

import concourse.bass as bass
import concourse.mybir as mybir
from concourse.tile import TileContext
from concourse.bass_utils import run_bass_kernel_spmd

F32 = mybir.dt.float32
BF16 = mybir.dt.bfloat16
BF = ml_dtypes.bfloat16

B, N_IN, N_OUT, C, OUT_C, KW = 8, 512, 1024, 16, 32, 5
N_CORES = 8
NT = N_IN // 128   # n tiles (4)
MH = N_OUT // 512  # m halves (2)
MT = 512 // 128    # m tiles per half (4)
ROWS = C * KW + 1  # im2col rows (81)
BAND_T = 16.0      # skip a block when a * gap^2 > BAND_T (exp < 1.2e-7)


# --- walrus workaround -----------------------------------------------------
# This container's walrus accepts at most ONE semaphore wait per TPB
# instruction, but Tile's scheduler attaches several (joins + tail drain).
# Hoist all but the last wait of each instruction onto fresh wait-only
# EventSemaphore instructions inserted right before it on the same engine.
_ws_ctr = [0]


def _split_multi_waits(nc):
    for fn in nc.m.functions:
        for blk in fn.blocks:
            insts = blk.instructions
            if not any(
                ins.sync_info and len(ins.sync_info.on_wait) > 1 for ins in insts
            ):
                continue
            out = []
            for ins in insts:
                si = ins.sync_info
                waits = list(si.on_wait) if si else []
                if len(waits) > 1:
                    for w in waits[:-1]:
                        _ws_ctr[0] += 1
                        ev = mybir.InstEventSemaphore(
                            name=f"waitsplit_{_ws_ctr[0]}", ins=[], outs=[]
                        )
                        ev.engine = ins.engine
                        ev.sync_info = mybir.SyncInfo(on_wait=[w], on_update=[])
                        nc.register_instruction(ev)
                        out.append(ev)
                    ins.sync_info = mybir.SyncInfo(
                        on_wait=[waits[-1]], on_update=list(si.on_update)
                    )
                out.append(ins)
            insts[:] = out


# --- kernel build ----------------------------------------------------------
def _build(key):
    """key: (groups, active) with groups ((c0, c1, a), ...) and active a
    tuple over n-tiles of frozensets of active m-halves."""
    groups, active = key
    single = len(groups) == 1
    nc = bass.Bass()
    # stack carries the im2col block (cols 0:512), conv weights
    # (cols 512:528) and linear weights (cols 528:560, rows 0:17)
    stack_d = nc.dram_tensor("stack", [ROWS, N_IN + C + OUT_C], BF16,
                             kind="ExternalInput")
    uv_d = nc.dram_tensor("uv", [8, N_IN + N_OUT], BF16, kind="ExternalInput")
    y_d = nc.dram_tensor("y", [N_OUT, OUT_C], F32, kind="ExternalOutput")

    Exp = mybir.ActivationFunctionType.Exp
    # active m-col range per n-tile (contiguous since halves are sorted)
    lo_hi = []
    for k in range(NT):
        mhs = sorted(active[k])
        lo_hi.append((mhs[0] * 512, (mhs[-1] + 1) * 512) if mhs else None)
    # active n-tiles per m-half
    ks_of = [
        [k for k in range(NT) if mh in active[k]] for mh in range(MH)
    ]

    with TileContext(nc) as tc:
        with (
            tc.tile_pool(name="const", bufs=1) as cpool,
            tc.tile_pool(name="work", bufs=1) as wpool,
            tc.tile_pool(name="psum", bufs=1, space="PSUM") as ppool,
        ):
            # --- warm exp: trigger the ACT table load at t~0 (no DMA dep) --
            wsrc = cpool.tile([8, 640], BF16)
            nc.vector.memset(wsrc[:], 0.0)
            wact = cpool.tile([8, 16], F32)
            nc.scalar.activation(wact[:], wsrc[:, 0:16], Exp)

            # --- input DMAs --------------------------------------------
            # uv gates the d2 pipeline: first on the sync queue.  Row-tiled
            # d2 matmuls need the operands at partition base 32k, so the 8
            # uv rows are replicated to 4 partition offsets (the k=0 copy
            # is the only one on the critical path).
            uvsb = cpool.tile([104, N_IN + N_OUT], BF16)
            nc.sync.dma_start(out=uvsb[0:8, :], in_=uv_d[:])
            nc.gpsimd.dma_start(out=uvsb[32:40, :], in_=uv_d[:])
            stack = cpool.tile([ROWS, N_IN + C + OUT_C], BF16)
            nc.gpsimd.dma_start(out=stack[:], in_=stack_d[:])
            nc.sync.dma_start(out=uvsb[64:72, :], in_=uv_d[:])
            nc.gpsimd.dma_start(out=uvsb[96:104, :], in_=uv_d[:])
            wa = stack[:, N_IN:N_IN + C]
            wl = stack[0:C + 1, N_IN + C:N_IN + C + OUT_C]

            # --- PE warmups: cover the DMA-latency dead zone so the HAM
            # clock gate can release and the queue isn't idle.
            wps = ppool.tile([128, 512], F32, tag="small", bufs=2)
            for i in range(3):
                nc.tensor.matmul(
                    wps[:],
                    lhsT=wsrc[:, 0:128],
                    rhs=wsrc[:, 128:640],
                    start=True,
                    stop=True,
                )

            def uL(k):   # (8, 128) d2 lhsT rows for n-tile k, at base 32k
                return uvsb[32 * k:32 * k + 8, k * 128:(k + 1) * 128]

            def vR(k, mh):  # (8, 512) d2 rhs rows at base 32k
                return uvsb[32 * k:32 * k + 8,
                            N_IN + mh * 512:N_IN + (mh + 1) * 512]

            # --- d2 matmuls (row-tiled) + exp per n-tile -------------------
            dsq = [
                ppool.tile([128, N_OUT], F32, tag="dsq", bufs=2,
                           name=f"dsq{k}")
                for k in range(NT)
            ]
            esb = {}
            for k in range(NT):
                if lo_hi[k] is None:
                    continue
                lo, hi = lo_hi[k]
                for mh in sorted(active[k]):
                    nc.tensor.matmul(
                        dsq[k][:, mh * 512:(mh + 1) * 512],
                        lhsT=uL(k),
                        rhs=vR(k, mh),
                        start=True,
                        stop=True,
                        tile_position=(32 * k, 0),
                    )
                for gi, (c0, c1, ag) in enumerate(groups):
                    e = wpool.tile([128, N_OUT], BF16, tag="esb",
                                   bufs=NT if single else 6,
                                   name=f"e{k}_{gi}")
                    nc.scalar.activation(e[:, lo:hi], dsq[k][:, lo:hi], Exp,
                                         scale=-float(ag))
                    esb[(k, gi)] = e

            # --- conv im2col matmuls --------------------------------------
            cps = ppool.tile([128, 4 * C], F32, tag="small", bufs=2)
            for k in range(NT):
                nc.tensor.matmul(
                    cps[:, k * C:(k + 1) * C],
                    lhsT=stack[:, k * 128:(k + 1) * 128],
                    rhs=wa,
                    start=True,
                    stop=True,
                )
            rsb = cpool.tile([128, 4 * C], BF16)
            nc.vector.tensor_copy(out=rsb[:], in_=cps[:])

            # --- z accumulation over active n-tiles per m-half ------------
            if single:
                # separate PSUM banks per m-half so the two zc drains can
                # run on DVE + ACT in parallel; mh1 sits at partition 32
                # (col group 1) so the per-n-tile matmul pairs overlap.
                zpsA = ppool.tile([16, 512], F32, tag="zpsA", bufs=1)
                zpsB = ppool.tile([48, 512], F32, tag="zpsB", bufs=1)
                zsl = [zpsA[:], zpsB[32:48, :]]
                for k in range(NT):
                    for mh in range(MH):
                        if mh not in active[k]:
                            continue
                        nc.tensor.matmul(
                            zsl[mh],
                            lhsT=rsb[:, k * C:k * C + 16],
                            rhs=esb[(k, 0)][:, mh * 512:(mh + 1) * 512],
                            start=(k == ks_of[mh][0]),
                            stop=(k == ks_of[mh][-1]),
                            tile_position=(0, 32 * mh),
                        )
            else:
                zps_g = {}
                for gi, (c0, c1, ag) in enumerate(groups):
                    for mh in range(MH):
                        zps_g[(gi, mh)] = ppool.tile(
                            [c1 - c0, 512], F32, tag=f"zps{mh}", bufs=1,
                            name=f"zps{mh}_{gi}")
                for k in range(NT):
                    for gi, (c0, c1, ag) in enumerate(groups):
                        for mh in range(MH):
                            if mh not in active[k]:
                                continue
                            nc.tensor.matmul(
                                zps_g[(gi, mh)][:],
                                lhsT=rsb[:, k * C + c0:k * C + c1],
                                rhs=esb[(k, gi)][:, mh * 512:(mh + 1) * 512],
                                start=(k == ks_of[mh][0]),
                                stop=(k == ks_of[mh][-1]),
                            )

            # --- zc: z in SBUF bf16 with a ones row for the bias ----------
            # memset ALL 17 rows to 1.0 (engine APs need base partition 0);
            # the z copies overwrite rows 0..15, leaving row 16 = ones,
            # which pairs with lin_b in wl's last row.
            zc = []
            for mh in range(MH):
                z = cpool.tile([C + 1, 512], BF16, name=f"zc{mh}")
                nc.vector.memset(z[:], 1.0)
                zc.append(z)
            zeng = [nc.vector, nc.scalar]
            for mh in range(MH):
                if single:
                    if mh == 0:
                        nc.vector.tensor_copy(out=zc[0][0:16, :], in_=zsl[0])
                    else:
                        nc.scalar.copy(out=zc[1][0:16, :], in_=zsl[1])
                else:
                    # general path: stage each group at partition 0, then
                    # SBUF->SBUF DMA into its channel rows (engines cannot
                    # address partition bases outside {0,32,64,96}).
                    for gi, (c0, c1, ag) in enumerate(groups):
                        stg = wpool.tile([c1 - c0, 512], BF16, tag="zstg",
                                         bufs=2, name=f"zstg{mh}_{gi}")
                        nc.vector.tensor_copy(out=stg[:],
                                              in_=zps_g[(gi, mh)][:])
                        nc.gpsimd.dma_start(out=zc[mh][c0:c1, :], in_=stg[:])

            # --- final linear + per-half contiguous stores ----------------
            # osb_mh[p, mt*32 + o] -> y row mh*512 + 4p + mt (device
            # layout; host inverse-gathers back to original xt order).
            # Each half's 64KB store issues as soon as its drain lands.
            oeng = [nc.sync, nc.gpsimd]
            for mh in range(MH):
                lps = ppool.tile([128, MT * OUT_C], F32, tag="small", bufs=2,
                                 name=f"lps{mh}")
                for mt in range(MT):
                    nc.tensor.matmul(
                        lps[:, mt * OUT_C:(mt + 1) * OUT_C],
                        lhsT=zc[mh][:, mt * 128:(mt + 1) * 128],
                        rhs=wl,
                        start=True,
                        stop=True,
                    )
                osb = wpool.tile([128, MT * OUT_C], F32, tag="osb", bufs=2,
                                 name=f"osb{mh}")
                if mh == 0:
                    nc.vector.tensor_copy(out=osb[:], in_=lps[:])
                else:
                    nc.scalar.copy(out=osb[:], in_=lps[:])
                y_r = y_d[mh * 512:(mh + 1) * 512, :].rearrange(
                    "(p j) o -> p j o", p=128
                )
                oeng[mh].dma_start(out=y_r, in_=osb[:])

    _split_multi_waits(nc)
    return nc


_cache = {}


def _get_nc(key):
    if key not in _cache:
        _cache[key] = _build(key)
    return _cache[key]


def _hi_lo(x):
    """Split fp64 array into bf16 hi + bf16 lo with x ~ hi + lo."""
    hi = x.astype(BF)
    lo = (x - hi.astype(np.float64)).astype(BF)
    return hi, lo


def _prepare(r, x_context, y_context, x_target, conv_w, conv_b, sigma, lin_w,
             lin_b):
    r = np.asarray(r, np.float64)
    x_context = np.asarray(x_context, np.float64)
    x_target = np.asarray(x_target, np.float64)
    conv_w = np.asarray(conv_w, np.float64)
    conv_b = np.asarray(conv_b, np.float64)
    sigma = np.asarray(sigma, np.float64)
    lin_w = np.asarray(lin_w, np.float64)
    lin_b = np.asarray(lin_b, np.float64)

    # Channels sharing a length scale share one RBF map: sort channels by a,
    # group runs of equal values (uniform init sigma -> a single group).
    scales = np.exp(sigma)
    a = 0.5 / scales**2
    perm = np.argsort(a, kind="stable")
    a_s = a[perm]
    groups = []
    c0 = 0
    for c in range(1, C + 1):
        if c == C or a_s[c] != a_s[c0]:
            groups.append((c0, c, float(a_s[c0])))
            c0 = c
    groups = tuple(groups)
    a_min = a_s[0]

    # conv weights (channel-permuted), bias row first to pair with the
    # ones row of the im2col stack.
    w_aug = np.concatenate(
        [conv_b[None, :], conv_w.transpose(2, 1, 0).reshape(C * KW, C)], axis=0
    )[:, perm].astype(BF)
    # linear weights with lin_b as the last row (pairs with zc's ones row)
    wl = np.concatenate([lin_w.T[perm], lin_b[None, :]], axis=0).astype(BF)

    pad = KW // 2
    in_maps = []
    act_sets = [set() for _ in range(NT)]
    gathers = []
    for b in range(B):
        u_raw = x_context[b, :, 0]
        v_raw = x_target[b, :, 0]
        u_idx = np.argsort(u_raw, kind="stable")
        v_idx = np.argsort(v_raw, kind="stable")
        u = u_raw[u_idx]
        v = v_raw[v_idx]

        # banding mask: block (n-tile k, m-half mh) active iff the sorted
        # value ranges come within gap, a*gap^2 <= BAND_T
        for k in range(NT):
            ulo, uhi = u[k * 128], u[(k + 1) * 128 - 1]
            for mh in range(MH):
                vlo, vhi = v[mh * 512], v[(mh + 1) * 512 - 1]
                gap = max(0.0, max(ulo - vhi, vlo - uhi))
                if a_min * gap * gap <= BAND_T:
                    act_sets[k].add(mh)

        # host im2col: ones row + 5 shifted copies of r (pure layout),
        # then permute columns into sorted-xc order; conv + linear
        # weights ride along in the trailing columns (one DMA).
        stack = np.zeros((ROWS, N_IN), np.float64)
        stack[0] = 1.0
        rb = r[b]
        for k in range(KW):
            lo = max(0, pad - k)
            hi = min(N_IN, N_IN + pad - k)
            stack[1 + C * k:1 + C * (k + 1), lo:hi] = rb[:, lo + k - pad:hi + k - pad]
        stack = stack[:, u_idx].astype(BF)
        tail = np.zeros((ROWS, C + OUT_C), BF)
        tail[:, 0:C] = w_aug
        tail[0:C + 1, C:C + OUT_C] = wl
        stack = np.concatenate([stack, tail], axis=1)

        # d2 factor rows: d2 = u^2 - 2uv + v^2 with exact bf16 products
        uh, ul = _hi_lo(u)
        vh, vl = _hi_lo(v)
        suh, sul = _hi_lo(u * u)
        svh, svl = _hi_lo(v * v)
        one_n = np.ones(N_IN, BF)
        one_m = np.ones(N_OUT, BF)
        zero_n = np.zeros(N_IN, BF)
        zero_m = np.zeros(N_OUT, BF)
        uL = np.stack([suh, sul,
                       (-2.0 * uh.astype(np.float64)).astype(BF),
                       (-2.0 * ul.astype(np.float64)).astype(BF),
                       (-2.0 * uh.astype(np.float64)).astype(BF),
                       one_n, one_n, zero_n])
        vR = np.stack([one_m, one_m, vh, vh, vl, svh, svl, zero_m])
        uv = np.concatenate([uL, vR], axis=1)

        in_maps.append({
            "stack": np.ascontiguousarray(stack),
            "uv": np.ascontiguousarray(uv),
        })

        # device row for sorted rank rk (= mh*512 + mt*128 + p) is
        # mh*512 + 4p + mt; output row v_idx[rk] gets device row devrow(rk)
        rk = np.arange(N_OUT)
        mh_, mt_, p_ = rk // 512, (rk // 128) % 4, rk % 128
        devrow = mh_ * 512 + 4 * p_ + mt_
        inv = np.empty(N_OUT, np.int64)
        inv[v_idx] = devrow
        gathers.append(inv)

    active = tuple(frozenset(s) for s in act_sets)
    key = (groups, active)
    return key, in_maps, gathers


def _assemble(res, gathers):
    return np.stack(
        [res.results[b]["y"][gathers[b]] for b in range(B)], axis=0
    )


def kernel(**inputs):
    key, in_maps, gathers = _prepare(**inputs)
    nc = _get_nc(key)
    res = run_bass_kernel_spmd(nc, in_maps, list(range(N_CORES)))
    return _assemble(res, gathers)


# revision 22
# speedup vs baseline: 1.9278x; 1.0684x over previous
"""ConvDecoder Bass kernel for Trainium2, SPMD over 8 NeuronCores.

Math (per batch element b, one per core):
    r_conv = Conv1d(r, conv_w, SAME) + conv_b            # (C, N_IN)
    d[n,m] = (xc[n] - xt[m])^2                           # (N_IN, N_OUT)
    E_c    = exp(-a_c * d),  a_c = 0.5 / exp(sigma_c)^2
    z[m,c] = sum_n r_conv[c,n] * E_c[n,m]
    out    = z @ lin_w.T + lin_b                         # (N_OUT, OUT_C)

Per-core structure (v7):
  - The pointwise linear is FOLDED INTO THE CONV WEIGHTS on host:
    W2_g = w_aug[:, c_g] @ lin_w.T[c_g] (+ lin_b on the im2col ones
    row), one block per length-scale group g.  The conv matmul then
    yields R2[n, o] = sum_c r_conv[n, c] lin_w[o, c] directly, and
    the z matmul produces y^T = sum_n R2[n,:]^T E[n,:] -- the output
    itself in (OUT_C, m) layout.  No on-device linear stage at all;
    the host transposes (free) when assembling.
  - im2col stack built on HOST in sorted-xc order; W2 rides in its
    trailing columns (ONE input DMA for all weights/data except uv).
  - d^2 on the PE as a rank-7 bf16 matmul (exact hi/lo splits), one
    row-tile position per n-tile so the matmuls overlap; ACT exp
    reads PSUM, writes bf16 E to SBUF.  ACT is the serial bottleneck.
  - BANDING: xc/xt sorted on host => far (n-tile, m-half) blocks are
    skipped (mask from the actual data, union over batches, baked
    into the compiled kernel).  With the usual 6/8 mask each dsq
    n-tile gets a DEDICATED PSUM slot (no rotation, d^2 never waits
    on ACT); the conv shares n-tile 0's bank after its exp drains.
  - y^T accumulates in two PSUM banks (m-half 0 -> partitions 0:32
    col group 0, m-half 1 -> 32:64 col group 1, concurrent), drained
    in parallel by DVE/ACT, stored by two contiguous 64KB DMAs in
    sorted order; host inverse-gathers rows.
"""

import numpy as np
import ml_dtypes

import concourse.bass as bass
import concourse.mybir as mybir
from concourse.tile import TileContext
from concourse.bass_utils import run_bass_kernel_spmd

F32 = mybir.dt.float32
BF16 = mybir.dt.bfloat16
BF = ml_dtypes.bfloat16

B, N_IN, N_OUT, C, OUT_C, KW = 8, 512, 1024, 16, 32, 5
N_CORES = 8
NT = N_IN // 128   # n tiles (4)
MH = N_OUT // 512  # m halves (2)
ROWS = C * KW + 1  # im2col rows (81)
BAND_T = 16.0      # skip a block when a * gap^2 > BAND_T (exp < 1.2e-7)


# --- walrus workaround -----------------------------------------------------
# This container's walrus accepts at most ONE semaphore wait per TPB
# instruction, but Tile's scheduler attaches several (joins + tail drain).
# Hoist all but the last wait of each instruction onto fresh wait-only
# EventSemaphore instructions inserted right before it on the same engine.
_ws_ctr = [0]


def _split_multi_waits(nc):
    for fn in nc.m.functions:
        for blk in fn.blocks:
            insts = blk.instructions
            if not any(
                ins.sync_info and len(ins.sync_info.on_wait) > 1 for ins in insts
            ):
                continue
            out = []
            for ins in insts:
                si = ins.sync_info
                waits = list(si.on_wait) if si else []
                if len(waits) > 1:
                    for w in waits[:-1]:
                        _ws_ctr[0] += 1
                        ev = mybir.InstEventSemaphore(
                            name=f"waitsplit_{_ws_ctr[0]}", ins=[], outs=[]
                        )
                        ev.engine = ins.engine
                        ev.sync_info = mybir.SyncInfo(on_wait=[w], on_update=[])
                        nc.register_instruction(ev)
                        out.append(ev)
                    ins.sync_info = mybir.SyncInfo(
                        on_wait=[waits[-1]], on_update=list(si.on_update)
                    )
                out.append(ins)
            insts[:] = out


# --- kernel build ----------------------------------------------------------
def _build(key):
    """key: (n_groups, a_tuple, active) - a_tuple the per-group exp scales,
    active a tuple over n-tiles of frozensets of active m-halves."""
    n_groups, a_tuple, active = key
    G = n_groups
    nc = bass.Bass()
    # stack carries the im2col block (cols 0:512) and the folded
    # conv+linear weights W2 per group (cols 512:512+32G)
    stack_d = nc.dram_tensor("stack", [ROWS, N_IN + G * OUT_C], BF16,
                             kind="ExternalInput")
    uv_d = nc.dram_tensor("uv", [8, N_IN + N_OUT], BF16, kind="ExternalInput")
    # output is y^T in sorted-m order; host transposes + inverse-gathers
    y_d = nc.dram_tensor("y", [OUT_C, N_OUT], F32, kind="ExternalOutput")

    Exp = mybir.ActivationFunctionType.Exp
    # active m-col range per n-tile (contiguous since halves are sorted)
    lo_hi = []
    for k in range(NT):
        mhs = sorted(active[k])
        lo_hi.append((mhs[0] * 512, (mhs[-1] + 1) * 512) if mhs else None)
    spans = [0 if lh is None else (lh[1] - lh[0]) // 512 for lh in lo_hi]
    ks_of = [[k for k in range(NT) if mh in active[k]] for mh in range(MH)]
    # dedicated dsq slots when the mask leaves room (needs sum of spans
    # + 2 z banks <= 8), else 2 rotating double-width slots
    dedicated = sum(spans) + 2 <= 8

    with TileContext(nc) as tc:
        with (
            tc.tile_pool(name="const", bufs=1) as cpool,
            tc.tile_pool(name="work", bufs=1) as wpool,
            tc.tile_pool(name="psum", bufs=1, space="PSUM") as ppool,
        ):
            # --- warm exp: trigger the ACT table load at t~0 (no DMA dep) --
            wsrc = cpool.tile([8, 640], BF16)
            nc.vector.memset(wsrc[:], 0.0)
            wact = cpool.tile([8, 16], F32)
            nc.scalar.activation(wact[:], wsrc[:, 0:16], Exp)

            # y^T accumulators: m-half 0 -> col group 0 of bank A,
            # m-half 1 -> col group 1 of bank B (concurrent matmuls AND
            # concurrent DVE/ACT drains)
            zA = ppool.tile([OUT_C, 512], F32, tag="zA", bufs=1)
            zB = ppool.tile([2 * OUT_C, 512], F32, tag="zB", bufs=1)
            zsl = [zA[:], zB[OUT_C:2 * OUT_C, :]]

            # --- input DMAs --------------------------------------------
            # uv gates the d2 pipeline: first on the sync queue.  Row-tiled
            # d2 matmuls need the operands at partition base 32k, so the 8
            # uv rows are replicated to 4 partition offsets (the k=0 copy
            # is the only one on the critical path).
            uvsb = cpool.tile([104, N_IN + N_OUT], BF16)
            nc.sync.dma_start(out=uvsb[0:8, :], in_=uv_d[:])
            nc.gpsimd.dma_start(out=uvsb[32:40, :], in_=uv_d[:])
            stack = cpool.tile([ROWS, N_IN + G * OUT_C], BF16)
            nc.gpsimd.dma_start(out=stack[:], in_=stack_d[:])
            nc.sync.dma_start(out=uvsb[64:72, :], in_=uv_d[:])
            nc.gpsimd.dma_start(out=uvsb[96:104, :], in_=uv_d[:])

            # --- PE warmups: cover the DMA-latency dead zone (they write
            # the unused low partitions of the zB bank)
            for i in range(2):
                nc.tensor.matmul(
                    zB[0:32, :],
                    lhsT=wsrc[:, 0:32],
                    rhs=wsrc[:, 128:640],
                    start=True,
                    stop=True,
                )

            def uL(k):   # (8, 128) d2 lhsT rows for n-tile k, at base 32k
                return uvsb[32 * k:32 * k + 8, k * 128:(k + 1) * 128]

            def vR(k, mh):  # (8, 512) d2 rhs rows at base 32k
                return uvsb[32 * k:32 * k + 8,
                            N_IN + mh * 512:N_IN + (mh + 1) * 512]

            # --- d2 matmuls (row-tiled) + exp per n-tile -------------------
            dsq = []
            for k in range(NT):
                if dedicated:
                    t = ppool.tile([128, spans[k] * 512 or 512], F32,
                                   tag=f"dsq{k}", bufs=1, name=f"dsq{k}")
                else:
                    t = ppool.tile([128, N_OUT], F32, tag="dsq", bufs=2,
                                   name=f"dsq{k}")
                dsq.append(t)
            esb = {}
            for k in range(NT):
                if lo_hi[k] is None:
                    continue
                lo, hi = lo_hi[k]
                for mh in sorted(active[k]):
                    nc.tensor.matmul(
                        dsq[k][:, mh * 512 - lo:(mh + 1) * 512 - lo],
                        lhsT=uL(k),
                        rhs=vR(k, mh),
                        start=True,
                        stop=True,
                        tile_position=(32 * k, 0),
                    )
                for gi in range(G):
                    e = wpool.tile([128, hi - lo], BF16, tag="esb",
                                   bufs=NT if G == 1 else NT + 2,
                                   name=f"e{k}_{gi}")
                    nc.scalar.activation(e[:], dsq[k][:, 0:hi - lo], Exp,
                                         scale=-float(a_tuple[gi]))
                    esb[(k, gi)] = e

            # --- conv matmuls: R2[n, 32g+o] = (r_conv @ lin_w.T)[n, o] ----
            # reuses n-tile 0's dsq bank once its exp has drained
            cps = ppool.tile(
                [128, min(G, 4) * NT * OUT_C], F32,
                tag="dsq0" if dedicated else "dsq",
                bufs=1 if dedicated else 2, name="cps")
            assert G <= 4, "more than 4 length-scale groups unsupported"
            for k in range(NT):
                nc.tensor.matmul(
                    cps[:, k * G * OUT_C:(k + 1) * G * OUT_C],
                    lhsT=stack[:, k * 128:(k + 1) * 128],
                    rhs=stack[0:ROWS, N_IN:N_IN + G * OUT_C],
                    start=True,
                    stop=True,
                )
            rsb = cpool.tile([128, NT * G * OUT_C], BF16)
            nc.vector.tensor_copy(out=rsb[:], in_=cps[:])

            # --- y^T accumulation over active (n-tile, group) -------------
            for k in range(NT):
                if lo_hi[k] is None:
                    continue
                lo, _ = lo_hi[k]
                for gi in range(G):
                    for mh in range(MH):
                        if mh not in active[k]:
                            continue
                        nc.tensor.matmul(
                            zsl[mh],
                            lhsT=rsb[:, (k * G + gi) * OUT_C:
                                     (k * G + gi + 1) * OUT_C],
                            rhs=esb[(k, gi)][:, mh * 512 - lo:
                                             (mh + 1) * 512 - lo],
                            start=(k == ks_of[mh][0] and gi == 0),
                            stop=(k == ks_of[mh][-1] and gi == G - 1),
                            tile_position=(0, 32 * mh),
                        )

            # --- parallel drains + two contiguous 64KB stores -------------
            osbA = wpool.tile([OUT_C, 512], F32, tag="osbA", bufs=1)
            nc.vector.tensor_copy(out=osbA[:], in_=zsl[0])
            nc.sync.dma_start(out=y_d[:, 0:512], in_=osbA[:])
            osbB = wpool.tile([OUT_C, 512], F32, tag="osbB", bufs=1)
            nc.scalar.copy(out=osbB[:], in_=zsl[1])
            nc.gpsimd.dma_start(out=y_d[:, 512:1024], in_=osbB[:])

    _split_multi_waits(nc)
    return nc


_cache = {}


def _get_nc(key):
    if key not in _cache:
        _cache[key] = _build(key)
    return _cache[key]


def _hi_lo(x):
    """Split fp64 array into bf16 hi + bf16 lo with x ~ hi + lo."""
    hi = x.astype(BF)
    lo = (x - hi.astype(np.float64)).astype(BF)
    return hi, lo


def _prepare(r, x_context, y_context, x_target, conv_w, conv_b, sigma, lin_w,
             lin_b):
    r = np.asarray(r, np.float64)
    x_context = np.asarray(x_context, np.float64)
    x_target = np.asarray(x_target, np.float64)
    conv_w = np.asarray(conv_w, np.float64)
    conv_b = np.asarray(conv_b, np.float64)
    sigma = np.asarray(sigma, np.float64)
    lin_w = np.asarray(lin_w, np.float64)
    lin_b = np.asarray(lin_b, np.float64)

    # Channels sharing a length scale share one RBF map: sort channels by a,
    # group runs of equal values (uniform init sigma -> a single group).
    scales = np.exp(sigma)
    a = 0.5 / scales**2
    perm = np.argsort(a, kind="stable")
    a_s = a[perm]
    groups = []
    c0 = 0
    for c in range(1, C + 1):
        if c == C or a_s[c] != a_s[c0]:
            groups.append((c0, c, float(a_s[c0])))
            c0 = c
    a_min = a_s[0]
    G = len(groups)

    # conv weights (channel-permuted), bias row first to pair with the
    # ones row of the im2col stack; FOLD the linear into them: per group
    # W2_g = w_aug[:, c_g] @ lin_w.T[c_g], with lin_b added on the ones
    # row of group 0.
    w_aug = np.concatenate(
        [conv_b[None, :], conv_w.transpose(2, 1, 0).reshape(C * KW, C)], axis=0
    )[:, perm]
    lw = lin_w.T[perm]  # (C, OUT_C), rows in permuted channel order
    w2 = np.zeros((ROWS, G * OUT_C), np.float64)
    for gi, (c0g, c1g, ag) in enumerate(groups):
        w2[:, gi * OUT_C:(gi + 1) * OUT_C] = w_aug[:, c0g:c1g] @ lw[c0g:c1g]
    w2[0, 0:OUT_C] += lin_b
    w2 = w2.astype(BF)

    pad = KW // 2
    in_maps = []
    act_sets = [set() for _ in range(NT)]
    gathers = []
    for b in range(B):
        u_raw = x_context[b, :, 0]
        v_raw = x_target[b, :, 0]
        u_idx = np.argsort(u_raw, kind="stable")
        v_idx = np.argsort(v_raw, kind="stable")
        u = u_raw[u_idx]
        v = v_raw[v_idx]

        # banding mask: block (n-tile k, m-half mh) active iff the sorted
        # value ranges come within gap, a*gap^2 <= BAND_T
        for k in range(NT):
            ulo, uhi = u[k * 128], u[(k + 1) * 128 - 1]
            for mh in range(MH):
                vlo, vhi = v[mh * 512], v[(mh + 1) * 512 - 1]
                gap = max(0.0, max(ulo - vhi, vlo - uhi))
                if a_min * gap * gap <= BAND_T:
                    act_sets[k].add(mh)

        # host im2col: ones row + 5 shifted copies of r (pure layout),
        # then permute columns into sorted-xc order; folded weights ride
        # along in the trailing columns (one DMA).
        stack = np.zeros((ROWS, N_IN), np.float64)
        stack[0] = 1.0
        rb = r[b]
        for k in range(KW):
            lo = max(0, pad - k)
            hi = min(N_IN, N_IN + pad - k)
            stack[1 + C * k:1 + C * (k + 1), lo:hi] = rb[:, lo + k - pad:hi + k - pad]
        stack = np.concatenate([stack[:, u_idx].astype(BF), w2], axis=1)

        # d2 factor rows: d2 = u^2 - 2uv + v^2 with exact bf16 products
        uh, ul = _hi_lo(u)
        vh, vl = _hi_lo(v)
        suh, sul = _hi_lo(u * u)
        svh, svl = _hi_lo(v * v)
        one_n = np.ones(N_IN, BF)
        one_m = np.ones(N_OUT, BF)
        zero_n = np.zeros(N_IN, BF)
        zero_m = np.zeros(N_OUT, BF)
        uL = np.stack([suh, sul,
                       (-2.0 * uh.astype(np.float64)).astype(BF),
                       (-2.0 * ul.astype(np.float64)).astype(BF),
                       (-2.0 * uh.astype(np.float64)).astype(BF),
                       one_n, one_n, zero_n])
        vR = np.stack([one_m, one_m, vh, vh, vl, svh, svl, zero_m])
        uv = np.concatenate([uL, vR], axis=1)

        in_maps.append({
            "stack": np.ascontiguousarray(stack),
            "uv": np.ascontiguousarray(uv),
        })

        # device column = sorted rank; host maps back to original order
        inv = np.empty(N_OUT, np.int64)
        inv[v_idx] = np.arange(N_OUT)
        gathers.append(inv)

    active = tuple(frozenset(s) for s in act_sets)
    key = (G, tuple(float(g[2]) for g in groups), active)
    return key, in_maps, gathers


def _assemble(res, gathers):
    return np.stack(
        [res.results[b]["y"].T[gathers[b]] for b in range(B)], axis=0
    )


def kernel(**inputs):
    key, in_maps, gathers = _prepare(**inputs)
    nc = _get_nc(key)
    res = run_bass_kernel_spmd(nc, in_maps, list(range(N_CORES)))
    return _assemble(res, gathers)
